# revision 10
# baseline (speedup 1.0000x reference)
"""Two-layer LSTM (B=256, T=1024, I=128, H=64) + sigmoid head on 8 TRN2 cores.

Data-parallel over batch: each core handles 32 sequences. Per core:
  - x is transposed on-chip (PE identity matmuls) into xT[128, T, 32]
    (partition = input dim, free = (t, batch)).
  - Recurrence layout: partition = gate/hidden dim, free = (layer, batch).
    Layer 2 lags layer 1 by LAG=8 steps; both layers' gates live in the
    same PSUM banks so one activation instruction covers both.
  - Per 8-step chunk one N=256 matmul fills layer-1 input contributions
    into PSUM; per-step K=65 matmuls (ones row in the h rings folds the
    biases into the weights) accumulate the recurrent parts.
  - Output head: every 16 steps one K=65 matmul + sigmoid + strided DMA.
"""

import sys
import types
from contextlib import ExitStack

import numpy as np

for _p in ("/root/.axon_site/_ro/trn_rl_repo", "/opt/trn_rl_repo"):
    try:
        import concourse.bass  # noqa: F401

        break
    except ImportError:
        if _p not in sys.path:
            sys.path.insert(0, _p)

import concourse.bacc as bacc
import concourse.tile as tile
from concourse import mybir
from concourse.bass_utils import run_bass_kernel_spmd
from concourse.masks import make_identity

AF = mybir.ActivationFunctionType
F32 = mybir.dt.float32

NCORES = 8
B_CORE = 32
T_FULL = 1024
I_IN = 128
H = 64
CH = 8  # steps per PSUM chunk; also the layer-2 lag
ORING = 16  # output-head ring length


def build_nc(Tv=T_FULL):
    assert Tv % ORING == 0 and Tv % 128 == 0 or Tv < 128
    nc = bacc.Bacc("TRN2", target_bir_lowering=False)
    x = nc.declare_dram_parameter("x", [B_CORE, Tv, I_IN], F32, isOutput=False)
    wl1x = nc.declare_dram_parameter("wl1x", [I_IN, 2, 2 * H], F32, isOutput=False)
    wl1h = nc.declare_dram_parameter("wl1h", [H + 1, 2, 2 * H], F32, isOutput=False)
    wl2x = nc.declare_dram_parameter("wl2x", [H + 1, 2, 2 * H], F32, isOutput=False)
    wl2h = nc.declare_dram_parameter("wl2h", [H, 2, 2 * H], F32, isOutput=False)
    wout = nc.declare_dram_parameter("wout", [H + 1, 1], F32, isOutput=False)
    y = nc.declare_dram_parameter("y", [1, B_CORE, Tv], F32, isOutput=True)

    with tile.TileContext(nc) as tc, ExitStack() as ctx:
        wpool = ctx.enter_context(tc.tile_pool(name="w", bufs=1))
        state = ctx.enter_context(tc.tile_pool(name="state", bufs=1))
        xpool = ctx.enter_context(tc.tile_pool(name="xT", bufs=1))

        s_wl1x = wpool.tile([I_IN, 2, 2 * H], F32)
        s_wl1h = wpool.tile([H + 1, 2, 2 * H], F32)
        s_wl2x = wpool.tile([H + 1, 2, 2 * H], F32)
        s_wl2h = wpool.tile([H, 2, 2 * H], F32)
        s_wout = wpool.tile([H + 1, 1], F32)
        nc.sync.dma_start(out=s_wl1x[:], in_=wl1x[:])
        nc.sync.dma_start(out=s_wl1h[:], in_=wl1h[:])
        nc.sync.dma_start(out=s_wl2x[:], in_=wl2x[:])
        nc.sync.dma_start(out=s_wl2h[:], in_=wl2h[:])
        nc.sync.dma_start(out=s_wout[:], in_=wout[:])
        ident = wpool.tile([128, 128], F32)
        make_identity(nc, ident)

        xT = xpool.tile([I_IN, Tv, B_CORE], F32)
        h1r = state.tile([H + 1, CH, B_CORE], F32)
        h2r = state.tile([H + 1, ORING, B_CORE], F32)
        nc.vector.memset(h1r[:], 0.0)
        nc.vector.memset(h1r[H : H + 1, :, :], 1.0)
        nc.vector.memset(h2r[:], 0.0)
        nc.vector.memset(h2r[H : H + 1, :, :], 1.0)

        # ---- phase B: transpose x into xT ----
        tblk = min(Tv, 128)
        with (
            tc.tile_pool(name="xin", bufs=3) as xin_pool,
            tc.tile_pool(name="ptr", bufs=2, space="PSUM") as ptr_pool,
        ):
            for b in range(B_CORE):
                for tb in range(Tv // tblk):
                    xt = xin_pool.tile([tblk, I_IN], F32, tag="xt")
                    nc.sync.dma_start(
                        out=xt[:], in_=x[b, tb * tblk : (tb + 1) * tblk, :]
                    )
                    pt = ptr_pool.tile([I_IN, tblk], F32, tag="pt")
                    nc.tensor.transpose(pt[:], xt[:], ident[0:tblk, 0:tblk])
                    nc.vector.tensor_copy(
                        xT[:, tb * tblk : (tb + 1) * tblk, b], pt[:]
                    )

        # ---- phase C: recurrence ----
        gpool = ctx.enter_context(tc.tile_pool(name="g", bufs=3))
        cpool = ctx.enter_context(tc.tile_pool(name="c", bufs=2))
        tpool = ctx.enter_context(tc.tile_pool(name="tt", bufs=3))
        opool = ctx.enter_context(tc.tile_pool(name="ob", bufs=2))
        pgates = ctx.enter_context(tc.tile_pool(name="pg", bufs=3, space="PSUM"))
        pout = ctx.enter_context(tc.tile_pool(name="po", bufs=2, space="PSUM"))

        c_prev = cpool.tile([H, 2, B_CORE], F32, tag="c")
        nc.vector.memset(c_prev[:], 0.0)

        ifb = gob = None
        for k in range(Tv + CH):
            l1 = k < Tv
            l2 = k >= CH
            kl = k % CH
            s = k - CH
            if kl == 0:
                ifb = pgates.tile([128, 2, CH, B_CORE], F32, tag="ifb")
                gob = pgates.tile([128, 2, CH, B_CORE], F32, tag="gob")
                if l1:
                    nc.tensor.matmul(
                        ifb[:, 0], s_wl1x[:, 0, :], xT[:, k : k + CH, :],
                        start=True, stop=False, skip_group_check=True,
                    )
                    nc.tensor.matmul(
                        gob[:, 0], s_wl1x[:, 1, :], xT[:, k : k + CH, :],
                        start=True, stop=False, skip_group_check=True,
                    )
                else:
                    # poison the fresh bank once so lane-1 accumulates onto zeros
                    nc.tensor.matmul(
                        ifb[:, 0, 0, :], s_wl1x[:, 0, :], xT[:, 0, :],
                        start=True, stop=False, skip_group_check=True,
                    )
                    nc.tensor.matmul(
                        gob[:, 0, 0, :], s_wl1x[:, 1, :], xT[:, 0, :],
                        start=True, stop=False, skip_group_check=True,
                    )
            if l1:
                h1prev = h1r[:, (k - 1) % CH, :]
                nc.tensor.matmul(
                    ifb[:, 0, kl, :], s_wl1h[:, 0, :], h1prev,
                    start=False, stop=True, skip_group_check=True,
                )
                nc.tensor.matmul(
                    gob[:, 0, kl, :], s_wl1h[:, 1, :], h1prev,
                    start=False, stop=True, skip_group_check=True,
                )
            if l2:
                h1s = h1r[:, kl, :]
                h2prev = h2r[0:H, (s - 1) % ORING, :]
                nc.tensor.matmul(
                    ifb[:, 1, kl, :], s_wl2x[:, 0, :], h1s, start=False, stop=False, skip_group_check=True
                )
                nc.tensor.matmul(
                    gob[:, 1, kl, :], s_wl2x[:, 1, :], h1s, start=False, stop=False, skip_group_check=True
                )
                nc.tensor.matmul(
                    ifb[:, 1, kl, :], s_wl2h[:, 0, :], h2prev,
                    start=False, stop=True, skip_group_check=True,
                )
                nc.tensor.matmul(
                    gob[:, 1, kl, :], s_wl2h[:, 1, :], h2prev,
                    start=False, stop=True, skip_group_check=True,
                )

            if l1 and l2:
                li, ln = 0, 2
            elif l1:
                li, ln = 0, 1
            else:
                li, ln = 1, 1
            sl_ = slice(li, li + ln)

            si = gpool.tile([H, 2, B_CORE], F32, tag="si")
            sf = gpool.tile([H, 2, B_CORE], F32, tag="sf")
            gg = gpool.tile([H, 2, B_CORE], F32, tag="gg")
            oo = gpool.tile([H, 2, B_CORE], F32, tag="oo")
            nc.scalar.activation(si[:, sl_, :], ifb[0:H, sl_, kl, :], AF.Sigmoid)
            nc.scalar.activation(sf[:, sl_, :], ifb[H:, sl_, kl, :], AF.Sigmoid)
            nc.scalar.activation(gg[:, sl_, :], gob[0:H, sl_, kl, :], AF.Tanh)
            nc.scalar.activation(oo[:, sl_, :], gob[H:, sl_, kl, :], AF.Sigmoid)

            c_cur = cpool.tile([H, 2, B_CORE], F32, tag="c")
            ig = tpool.tile([H, 2, B_CORE], F32, tag="ig")
            nc.vector.tensor_mul(ig[:, sl_, :], si[:, sl_, :], gg[:, sl_, :])
            nc.vector.tensor_mul(
                c_cur[:, sl_, :], sf[:, sl_, :], c_prev[:, sl_, :]
            )
            nc.vector.tensor_add(c_cur[:, sl_, :], c_cur[:, sl_, :], ig[:, sl_, :])
            if k == CH - 1:
                # zero the layer-2 lane before its first real use next tick
                nc.vector.memset(c_cur[:, 1, :], 0.0)

            tct = tpool.tile([H, 2, B_CORE], F32, tag="tc")
            nc.scalar.activation(tct[:, sl_, :], c_cur[:, sl_, :], AF.Tanh)
            if l1:
                nc.vector.tensor_mul(h1r[0:H, k % CH, :], oo[:, 0, :], tct[:, 0, :])
            if l2:
                nc.vector.tensor_mul(
                    h2r[0:H, s % ORING, :], oo[:, 1, :], tct[:, 1, :]
                )
            c_prev = c_cur

            if l2 and s % ORING == ORING - 1:
                po = pout.tile([1, ORING, B_CORE], F32, tag="po")
                nc.tensor.matmul(po[:], s_wout[:], h2r[:], start=True, stop=True)
                ob = opool.tile([1, B_CORE, ORING], F32, tag="ob")
                nc.scalar.activation(
                    ob.rearrange("p b t -> p t b"), po[:], AF.Sigmoid
                )
                t0 = s - (ORING - 1)
                nc.sync.dma_start(out=y[:, :, t0 : t0 + ORING], in_=ob[:])

    nc.compile()
    return nc


def pack_weights(w_ih0, w_hh0, b_ih0, b_hh0, w_ih1, w_hh1, b_ih1, b_hh1, w_out, b_out):
    def blocks(m):  # [R, C] with R=2H rows per block -> [C, 2, 2H]
        mt = np.ascontiguousarray(m.T)
        return np.stack([mt[:, : 2 * H], mt[:, 2 * H :]], axis=1)

    b0 = (b_ih0 + b_hh0).astype(np.float32)
    b1 = (b_ih1 + b_hh1).astype(np.float32)
    wl1x = blocks(w_ih0)  # [128, 2, 128]
    wl1h = np.concatenate(
        [blocks(w_hh0), np.stack([b0[: 2 * H], b0[2 * H :]], axis=0)[None]], axis=0
    )  # [65, 2, 128]
    wl2x = np.concatenate(
        [blocks(w_ih1), np.stack([b1[: 2 * H], b1[2 * H :]], axis=0)[None]], axis=0
    )
    wl2h = blocks(w_hh1)  # [64, 2, 128]
    wo = np.concatenate([w_out.T, b_out[None, :]], axis=0)  # [65, 1]
    return dict(
        wl1x=np.ascontiguousarray(wl1x, np.float32),
        wl1h=np.ascontiguousarray(wl1h, np.float32),
        wl2x=np.ascontiguousarray(wl2x, np.float32),
        wl2h=np.ascontiguousarray(wl2h, np.float32),
        wout=np.ascontiguousarray(wo, np.float32),
    )


def _install_ntff_hook():
    try:
        import antenv.axon_hooks  # noqa: F401

        return
    except ImportError:
        pass
    try:
        sys.path.insert(0, "/root/.axon_site")
        from trn_agent_boot.trn_boot import _ntff_profile_via_ctypes

        hook = _ntff_profile_via_ctypes("/opt/axon/libaxon_pjrt.so")
        m = types.ModuleType("antenv.axon_hooks")
        m.get_axon_ntff_profile_hook = lambda: hook
        sys.modules["antenv.axon_hooks"] = m
    except Exception:
        pass


_NC_CACHE = {}


def run_on_hw(x_full, wmap, Tv=T_FULL, trace=False):
    if Tv not in _NC_CACHE:
        _NC_CACHE[Tv] = build_nc(Tv)
    nc = _NC_CACHE[Tv]
    in_maps = []
    for c in range(NCORES):
        m = {"x": np.ascontiguousarray(x_full[c * B_CORE : (c + 1) * B_CORE, :Tv, :])}
        m.update(wmap)
        in_maps.append(m)
    if trace:
        _install_ntff_hook()
    res = run_bass_kernel_spmd(nc, in_maps, list(range(NCORES)), trace=trace)
    out = np.concatenate([res.results[c]["y"][0] for c in range(NCORES)], axis=0)
    return out[..., None].astype(np.float32), res


def kernel(x, w_ih0, w_hh0, b_ih0, b_hh0, w_ih1, w_hh1, b_ih1, b_hh1, w_out, b_out):
    wmap = pack_weights(
        np.asarray(w_ih0), np.asarray(w_hh0), np.asarray(b_ih0), np.asarray(b_hh0),
        np.asarray(w_ih1), np.asarray(w_hh1), np.asarray(b_ih1), np.asarray(b_hh1),
        np.asarray(w_out), np.asarray(b_out),
    )
    out, _ = run_on_hw(np.asarray(x, np.float32), wmap, T_FULL, trace=False)
    return out
